# revision 10
# baseline (speedup 1.0000x reference)
"""Trainium2 Bass kernel for nn_Concatenation_90701119357422.

Computes, for full inputs:
    ret  = mean(ret_feat, axis=1) @ Wp.T + bp          # [B, H]
    out  = concat([h, ret[batch]], -1) @ Wl.T + bl     # [N, H]

Strategy (8 cores, data-parallel over N):
  - out = h @ Wl[:, :H].T + ret2[batch]  where  ret2 = ret @ Wl[:, H:].T + bl
  - host casts h to fp16 and pre-transposes it into two feature-major halves
    per core; device runs fp16 matmuls with fp32 PSUM accumulation
  - ret2 is computed on device from ret_feat (replicated), using a host-folded
    matrix A = (Wp.T/16) @ Wl[:, H:].T and c = bp @ Wl[:, H:].T + bl
  - per-row gather ret2[batch] is a one-hot matmul accumulated into the same
    PSUM tile; the one-hot is built on device from batch values (PE broadcast
    matmul + DVE is_equal); ret2 applied as fp16 hi+lo pair (near-fp32 exact)
"""

import os
import sys

import numpy as np

for _p in ("/opt/trn_rl_repo", "/root/.axon_site/_ro/trn_rl_repo"):
    if os.path.isdir(_p) and _p not in sys.path:
        sys.path.append(_p)

import concourse.bass as bass
import concourse.mybir as mybir
import concourse.tile as tile
from concourse import bacc
from concourse.bass_utils import run_bass_kernel_spmd

N_TOTAL = 262144
B = 64
K = 16
H = 256
R = 512
N_CORES = 8
SHARD = N_TOTAL // N_CORES  # 32768

CHUNK = 4096                 # rows per pipeline chunk
F32 = mybir.dt.float32
F16 = mybir.dt.float16


def build_program(shard_rows: int = SHARD):
    assert shard_rows % CHUNK == 0
    n_chunks = shard_rows // CHUNK
    tiles_per_chunk = CHUNK // 128

    nc = bacc.Bacc("TRN2", target_bir_lowering=False, debug=False)

    # feature-major fp16 h halves: hta[k, r] = h[r, k], htb[k, r] = h[r, 128+k]
    hta_d = nc.dram_tensor("hta", [128, shard_rows], F16, kind="ExternalInput").ap()
    htb_d = nc.dram_tensor("htb", [128, shard_rows], F16, kind="ExternalInput").ap()
    bt = nc.dram_tensor("bt", [1, shard_rows], F16, kind="ExternalInput").ap()
    wt16 = nc.dram_tensor("wt16", [H, H], F16, kind="ExternalInput").ap()
    r2hi_d = nc.dram_tensor("r2hi", [128, H], F16, kind="ExternalInput").ap()
    r2lo_d = nc.dram_tensor("r2lo", [128, H], F16, kind="ExternalInput").ap()
    out = nc.dram_tensor("out", [shard_rows, H], F32, kind="ExternalOutput").ap()

    iota128_dr = nc.inline_tensor(
        np.arange(128, dtype=np.float32).reshape(128, 1), "iota128"
    ).ap()

    with tile.TileContext(nc) as tc:
        with (
            tc.tile_pool(name="const", bufs=1) as cpool,
            tc.tile_pool(name="psum", bufs=1, space="PSUM") as ppool,
            tc.tile_pool(name="ht", bufs=3) as hpool,
            tc.tile_pool(name="oh", bufs=3) as ohpool,
            tc.tile_pool(name="outp", bufs=3) as opool,
        ):
            # ---- constants into SBUF ----
            wt_sb = cpool.tile([128, 2, H], F16)
            nc.scalar.dma_start(wt_sb[:], wt16.rearrange("(kc p) c -> p kc c", p=128))
            iota128_sb = cpool.tile([128, 1], F32)
            nc.scalar.dma_start(iota128_sb[:], iota128_dr[:])
            ret2hi = cpool.tile([128, H], F16)
            nc.scalar.dma_start(ret2hi[:], r2hi_d[:])
            ret2lo = cpool.tile([128, H], F16)
            nc.scalar.dma_start(ret2lo[:], r2lo_d[:])

            # ---- main loop ----
            for ci in range(n_chunks):
                r0 = ci * CHUNK
                hta = hpool.tile([128, CHUNK], F16, tag="hta")
                nc.sync.dma_start(out=hta[:], in_=hta_d[:, r0 : r0 + CHUNK])
                htb = hpool.tile([128, CHUNK], F16, tag="htb")
                nc.sync.dma_start(out=htb[:], in_=htb_d[:, r0 : r0 + CHUNK])
                bts = ohpool.tile([1, CHUNK], F16, tag="bts")
                nc.sync.dma_start(out=bts[:], in_=bt[0:1, r0 : r0 + CHUNK])

                oh = ohpool.tile([128, CHUNK], F16, tag="oh")
                for half in range(CHUNK // 512):
                    hsl = slice(512 * half, 512 * (half + 1))
                    bcb = ohpool.tile([128, 512], F16, tag="bcb", bufs=3)
                    nc.gpsimd.partition_broadcast(bcb[:], bts[0:1, hsl])
                    nc.vector.tensor_scalar(
                        oh[:, hsl],
                        bcb[:],
                        iota128_sb[:],
                        None,
                        mybir.AluOpType.is_equal,
                    )

                outsb = opool.tile([128, tiles_per_chunk, H], F32, tag="outsb", bufs=2)
                for t in range(tiles_per_chunk):
                    ps = ppool.tile([128, H], F32, tag="acc", bufs=8)
                    sl = slice(128 * t, 128 * (t + 1))
                    nc.tensor.matmul(
                        ps[:], hta[:, sl], wt_sb[:, 0], start=True, stop=False
                    )
                    nc.tensor.matmul(
                        ps[:], htb[:, sl], wt_sb[:, 1], start=False, stop=False
                    )
                    nc.tensor.matmul(
                        ps[:], oh[:, sl], ret2hi[:], start=False, stop=False
                    )
                    nc.tensor.matmul(
                        ps[:], oh[:, sl], ret2lo[:], start=False, stop=True
                    )
                    if t % 2 == 0:
                        nc.vector.tensor_copy(outsb[:, t], ps[:])
                    else:
                        nc.scalar.copy(outsb[:, t], ps[:])

                nc.scalar.dma_start(
                    out=out[r0 : r0 + CHUNK, :].rearrange("(t p) n -> p t n", p=128),
                    in_=outsb[:],
                )

    nc.compile()
    return nc


def prep_inputs(h, ret_feat, batch, Wp, bp, Wl, bl, shard_rows: int = SHARD,
                n_cores: int = N_CORES):
    """Host-side prep: shard + cast + pre-transpose h. Returns per-core maps."""
    h = np.asarray(h, dtype=np.float32)
    Wl = np.asarray(Wl, dtype=np.float32)
    Wp = np.asarray(Wp, dtype=np.float32)
    bp = np.asarray(bp, dtype=np.float32)
    bl = np.asarray(bl, dtype=np.float32)
    ret_feat = np.asarray(ret_feat, dtype=np.float32)

    h16 = h.astype(np.float16)
    bt_all = np.asarray(batch).astype(np.float16)

    wt16 = np.ascontiguousarray(Wl[:, :H].T).astype(np.float16)
    # replicated pooled ret table: ret2 = (mean_k rf) @ Wp.T + bp) @ Wl[:,H:].T + bl
    wlr_t = Wl[:, H:].astype(np.float64).T  # [R, H]
    ret = ret_feat.astype(np.float64).mean(axis=1) @ Wp.astype(np.float64).T + bp
    ret2 = ret @ wlr_t + bl  # [B, H] float64
    r2hi = np.zeros((128, H), dtype=np.float16)
    r2lo = np.zeros((128, H), dtype=np.float16)
    r2hi[:B] = ret2.astype(np.float16)
    r2lo[:B] = (ret2 - r2hi[:B].astype(np.float64)).astype(np.float16)

    in_maps = []
    for i in range(n_cores):
        s = slice(i * shard_rows, (i + 1) * shard_rows)
        hs = h16[s]
        in_maps.append(
            {
                "hta": np.ascontiguousarray(hs[:, :128].T),
                "htb": np.ascontiguousarray(hs[:, 128:].T),
                "bt": np.ascontiguousarray(bt_all[s].reshape(1, shard_rows)),
                "wt16": wt16,
                "r2hi": r2hi,
                "r2lo": r2lo,
            }
        )
    return in_maps


_PROGRAM_CACHE = {}


def _get_program(shard_rows: int = SHARD):
    if shard_rows not in _PROGRAM_CACHE:
        _PROGRAM_CACHE[shard_rows] = build_program(shard_rows)
    return _PROGRAM_CACHE[shard_rows]


def kernel(h, ret_feat, batch, Wp, bp, Wl, bl):
    nc = _get_program(SHARD)
    in_maps = prep_inputs(h, ret_feat, batch, Wp, bp, Wl, bl)
    res = run_bass_kernel_spmd(nc, in_maps, list(range(N_CORES)))
    return np.concatenate([res.results[i]["out"] for i in range(N_CORES)], axis=0)
